# revision 1
# baseline (speedup 1.0000x reference)
"""Trainium2 Bass kernel for 16-head causal MHA (B=2, T=4096, D=1024).

Sharding: 8 cores = 2 batches x 4 head-groups (4 heads each).
Each core computes Q/K/V projections for its 256 cols of Wq/Wk/Wv,
streaming causal attention for its 4 heads, and a partial output
projection against its 256 rows of Wo.  Host sums the 4 partials per
batch and adds the output bias.

Device layouts (everything "transposed", T on the free axis):
  xT   [1024, 4096] bf16  (x[b].T)
  Qt/Kt as 2 SBUF tensors [128, 4096] packing 2 heads (64 rows each)
  V    stored per 128-row T-chunk as [128, 4*65(+pad)] with a ones
       column appended per head ([V|1] trick: PV matmul row 64 = sum(P))
  S^T  computed per (head, 512-query-block, 128-key-chunk) as
       matmul(lhsT=Kt chunk [64,128], rhs=Qt block [64,512]) -> PSUM
  P    = exp(S/8) on ScalarE (PSUM fp32 -> SBUF bf16), causal masking of
       diagonal tiles via gpsimd affine_select
  ctx^T accumulated in PSUM ([V|1] stationary, P^T moving), normalized
       by 1/l with a gpsimd partition-broadcast of the row-64 sums
  out  o^T [1024, 4096] fp32 = Wo_slice^T @ ctx^T, host transposes+sums

Because attention for query block qb only attends to keys <= block qb,
projections and attention are fully interleaved per block:
  {K(tb), V(tb), Q(tb), attention(tb), outproj(tb-1)}
which keeps ScalarE (the exp bottleneck) busy from the start.
"""

import math

import numpy as np

B, T, D = 2, 4096, 1024
H, HD = 16, 64
NCORES = 8
HPC = 4               # heads per core
DQ = HPC * HD         # 256 per-core projection width
P = 128
TQB = 512             # query block
NQB = T // TQB        # 8
NDC = D // P          # 8 contraction chunks for projections
NTC = T // P          # 32 key/T chunks
VST = 328             # per-T-chunk V stride: 4 heads * 65 + 63 pad

_NC_CACHE = {}


def _build_nc(repeat=1, loop=1):
    import concourse.mybir as mybir
    from concourse import bacc
    from concourse.tile import TileContext

    dt = mybir.dt
    bf = dt.bfloat16
    f32 = dt.float32
    AF = mybir.ActivationFunctionType
    ALU = mybir.AluOpType

    nc = bacc.Bacc("TRN2", target_bir_lowering=False, debug=False)

    xT = nc.dram_tensor("xT", [D, T], bf, kind="ExternalInput")
    wq = nc.dram_tensor("wq", [D, DQ], bf, kind="ExternalInput")
    wk = nc.dram_tensor("wk", [D, DQ], bf, kind="ExternalInput")
    wv = nc.dram_tensor("wv", [D, DQ], bf, kind="ExternalInput")
    woa = nc.dram_tensor("woa", [P, D], bf, kind="ExternalInput")
    wob = nc.dram_tensor("wob", [P, D], bf, kind="ExternalInput")
    bqk = nc.dram_tensor("bqk", [P, 4], f32, kind="ExternalInput")
    bv1 = nc.dram_tensor("bv1", [1, DQ], bf, kind="ExternalInput")
    ot = nc.dram_tensor("ot", [D, T], f32, kind="ExternalOutput")

    with TileContext(nc) as tc:
        with (
            tc.tile_pool(name="per", bufs=1) as per,
            tc.tile_pool(name="xp", bufs=2) as xp,
            tc.tile_pool(name="ptp", bufs=5) as ptp,
            tc.tile_pool(name="smp", bufs=3) as smp,
            tc.tile_pool(name="obp", bufs=3) as obp,
            tc.tile_pool(name="psM", bufs=2, space="PSUM") as psM,
            tc.tile_pool(name="psC", bufs=1, space="PSUM") as psC,
            tc.tile_pool(name="psS", bufs=3, space="PSUM") as psS,
        ):
            # ---- persistent tensors ----
            qt = [
                per.tile([P, T], bf, tag=f"qt{i}", name=f"qt{i}") for i in range(2)
            ]
            kt = [
                per.tile([P, T], bf, tag=f"kt{i}", name=f"kt{i}") for i in range(2)
            ]
            ctxt = [
                per.tile([P, T], bf, tag=f"ctxt{i}", name=f"ctxt{i}")
                for i in range(2)
            ]
            vsb = per.tile([P, NTC * VST], bf, tag="vsb")
            ctxn = per.tile([P, NTC * DQ], bf, tag="ctxn")
            ident = per.tile([P, P], bf, tag="ident")
            wq_sb = per.tile([P, NDC * DQ], bf, tag="wq")
            wk_sb = per.tile([P, NDC * DQ], bf, tag="wk")
            wv_sb = per.tile([P, NDC * DQ], bf, tag="wv")
            woa_sb = per.tile([P, D], bf, tag="woa")
            wob_sb = per.tile([P, D], bf, tag="wob")
            bqk_sb = per.tile([P, 4], f32, tag="bqk")
            bv1_sb = per.tile([1, DQ], bf, tag="bv1")
            ones_sb = per.tile([1, P], bf, tag="ones")
            zrow_sb = per.tile([1, TQB], bf, tag="zrow")

            # ---- loads ----
            for w_sb, w_dr in ((wq_sb, wq), (wk_sb, wk), (wv_sb, wv)):
                nc.scalar.dma_start(
                    w_sb[:].rearrange("p (c n) -> p c n", c=NDC),
                    w_dr[:, :].rearrange("(c p) n -> p c n", p=P),
                )
            nc.scalar.dma_start(woa_sb[:], woa[:, :])
            nc.scalar.dma_start(wob_sb[:], wob[:, :])
            nc.sync.dma_start(bqk_sb[:], bqk[:, :])
            nc.sync.dma_start(bv1_sb[:], bv1[:, :])
            nc.vector.memset(ones_sb[:], 1.0)
            nc.vector.memset(zrow_sb[:], 0.0)
            # ones columns for the [V|1] trick (data copies overwrite the rest)
            nc.vector.memset(vsb[:], 1.0)
            nc.gpsimd.memset(ident[:], 0.0)
            nc.gpsimd.affine_select(
                out=ident[:], in_=ident[:],
                compare_op=ALU.not_equal, fill=1.0,
                base=0, pattern=[[-1, P]], channel_multiplier=1,
            )

            def load_xblk(tb):
                xblk = xp.tile([P, NDC * TQB], bf, tag="xblk", name="xblk")
                nc.sync.dma_start(
                    xblk[:].rearrange("p (c t) -> p c t", c=NDC),
                    xT[:, :].rearrange("(c p) t -> p c t", p=P)[
                        :, :, tb * TQB : (tb + 1) * TQB
                    ],
                )
                return xblk

            def proj_qk(xblk, tb, w_sb, dst, bcol):
                for j in range(2):
                    ps = psM.tile([P, TQB], f32, tag="mix", name="psqk")
                    for d in range(NDC):
                        nc.tensor.matmul(
                            ps[:],
                            w_sb[:, d * DQ + j * P : d * DQ + (j + 1) * P],
                            xblk[:, d * TQB : (d + 1) * TQB],
                            start=(d == 0),
                            stop=(d == NDC - 1),
                        )
                    nc.vector.tensor_scalar_add(
                        dst[j][:, tb * TQB : (tb + 1) * TQB],
                        ps[:],
                        bqk_sb[:, bcol + j : bcol + j + 1],
                    )

            def proj_v(xblk, tb):
                for t4 in range(4):
                    tc_ = tb * 4 + t4
                    ps = psM.tile([P, TQB], f32, tag="mix", name="psv")
                    for d in range(NDC):
                        nc.tensor.matmul(
                            ps[:, :DQ],
                            xblk[:, d * TQB + t4 * P : d * TQB + (t4 + 1) * P],
                            wv_sb[:, d * DQ : (d + 1) * DQ],
                            start=(d == 0),
                            stop=False,
                        )
                    nc.tensor.matmul(
                        ps[:, :DQ], ones_sb[:, :], bv1_sb[:, :],
                        start=False, stop=True,
                    )
                    dst = vsb[:, tc_ * VST : tc_ * VST + 4 * 65].rearrange(
                        "p (h d) -> p h d", h=HPC
                    )
                    nc.vector.tensor_copy(
                        dst[:, :, 0:64],
                        ps[:, :DQ].rearrange("p (h d) -> p h d", d=HD),
                    )

            # ctx accumulators: 16 regions (h*4+sq) of width 65 (col 64 = l)
            # packed 7-per-bank at 8-byte-aligned stride 66 -> 3 PSUM banks.
            # One accumulation group per bank: start on the first matmul that
            # touches the bank (region idx 0/7/14 at ck=0), stop on the last
            # (idx 6/13/15 at that region's diagonal chunk).
            def _creg(h, sq):
                idx = h * 4 + sq
                b, r = divmod(idx, 7)
                return b * 512 + r * 66

            def attention_chunk(qb, h, ck, psc):
                g2, off = h // 2, (h % 2) * 64
                nchunks = 4 * (qb + 1)
                tk0 = ck * P
                # columns of this query block that can see key chunk ck
                co = max(0, tk0 - qb * TQB)
                pss = psS.tile([P, 512], f32, tag="scores", name="pss")
                pt = ptp.tile([P, 512], bf, tag="pt", name="pt")
                nc.tensor.matmul(
                    pss[:, co:],
                    kt[g2][off : off + 64, tk0 : tk0 + P],
                    qt[g2][off : off + 64, qb * TQB + co : (qb + 1) * TQB],
                    start=True,
                    stop=True,
                )
                nc.scalar.activation(
                    pt[:, co:], pss[:, co:], AF.Exp, scale=1.0 / math.sqrt(HD)
                )
                if co > 0 or tk0 == qb * TQB:
                    # left-edge triangle: keep where p <= f' (local)
                    nc.gpsimd.affine_select(
                        pt[:, co : co + P], pt[:, co : co + P],
                        pattern=[[1, P]],
                        compare_op=ALU.is_ge,
                        fill=0.0,
                        base=0,
                        channel_multiplier=-1,
                    )
                for sq in range(co // P, 4):
                    idx = h * 4 + sq
                    r0 = _creg(h, sq)
                    nc.tensor.matmul(
                        psc[:, r0 : r0 + 65],
                        pt[:, sq * P : (sq + 1) * P],
                        vsb[:, ck * VST + h * 65 : ck * VST + (h + 1) * 65],
                        start=False,
                        stop=(
                            (idx == 6 and ck == 4 * qb + 2)
                            or (idx == 13 and ck == 4 * qb + 1)
                            or (idx == 15 and ck == 4 * qb + 3)
                        ),
                    )

            def normalize_head(qb, h, psc):
                # each query-sub accumulator: cols 0-63 ctx, col 64 = l
                for sq in range(4):
                    r0 = _creg(h, sq)
                    g = qb * 4 + sq
                    rsb = smp.tile([P, 1], f32, tag="r", name="rsb")
                    nc.vector.reciprocal(rsb[:], psc[:, r0 + 64 : r0 + 65])
                    nc.vector.tensor_scalar_mul(
                        ctxn[:, g * DQ + h * 64 : g * DQ + (h + 1) * 64],
                        psc[:, r0 : r0 + 64],
                        rsb[:, 0:1],
                    )

            def proj_tasks(tb):
                """Micro-tasks (~4 matmuls each) projecting block tb.
                Order: dma, Q j0, K j0, V t0..t3, Q j1, K j1 so attention on
                heads 0/1 can start as early as possible."""
                qk_tasks = {0: [], 1: []}
                v_tasks = []
                state = {}

                def get_xblk():
                    return state["xblk"]

                def t_dma():
                    state["xblk"] = load_xblk(tb)

                for j in range(2):
                    for w_sb, dst, bcol in ((wq_sb, qt, 0), (wk_sb, kt, 2)):

                        def t_a(w_sb=w_sb, j=j):
                            ps = psM.tile([P, TQB], f32, tag="mix", name="psqk")
                            state["ps"] = ps
                            for d in range(4):
                                nc.tensor.matmul(
                                    ps[:],
                                    w_sb[:, d * DQ + j * P : d * DQ + (j + 1) * P],
                                    get_xblk()[:, d * TQB : (d + 1) * TQB],
                                    start=(d == 0),
                                    stop=False,
                                )

                        def t_b(w_sb=w_sb, dst=dst, bcol=bcol, j=j):
                            ps = state["ps"]
                            for d in range(4, NDC):
                                nc.tensor.matmul(
                                    ps[:],
                                    w_sb[:, d * DQ + j * P : d * DQ + (j + 1) * P],
                                    get_xblk()[:, d * TQB : (d + 1) * TQB],
                                    start=False,
                                    stop=(d == NDC - 1),
                                )
                            nc.vector.tensor_scalar_add(
                                dst[j][:, tb * TQB : (tb + 1) * TQB],
                                ps[:],
                                bqk_sb[:, bcol + j : bcol + j + 1],
                            )

                        qk_tasks[j].append(t_a)
                        qk_tasks[j].append(t_b)
                for t4 in range(4):

                    def v_a(t4=t4):
                        ps = psM.tile([P, TQB], f32, tag="mix", name="psv")
                        state["ps"] = ps
                        for d in range(4):
                            nc.tensor.matmul(
                                ps[:, :DQ],
                                get_xblk()[
                                    :, d * TQB + t4 * P : d * TQB + (t4 + 1) * P
                                ],
                                wv_sb[:, d * DQ : (d + 1) * DQ],
                                start=(d == 0),
                                stop=False,
                            )

                    def v_b(t4=t4):
                        ps = state["ps"]
                        tc_ = tb * 4 + t4
                        for d in range(4, NDC):
                            nc.tensor.matmul(
                                ps[:, :DQ],
                                get_xblk()[
                                    :, d * TQB + t4 * P : d * TQB + (t4 + 1) * P
                                ],
                                wv_sb[:, d * DQ : (d + 1) * DQ],
                                start=False,
                                stop=False,
                            )
                        nc.tensor.matmul(
                            ps[:, :DQ], ones_sb[:, :], bv1_sb[:, :],
                            start=False, stop=True,
                        )
                        for hh in range(HPC):
                            nc.vector.tensor_copy(
                                vsb[
                                    :,
                                    tc_ * VST + hh * 65 : tc_ * VST + hh * 65 + 64,
                                ],
                                ps[:, hh * HD : (hh + 1) * HD],
                            )

                    v_tasks.append(v_a)
                    v_tasks.append(v_b)

                yield t_dma
                yield from qk_tasks[0]
                yield from v_tasks
                yield from qk_tasks[1]

            def transp_tasks(qb):
                """Transpose ctx blocks of qb for the output projection."""
                for sq in range(4):

                    def t(sq=sq):
                        g = qb * 4 + sq
                        for c in range(2):
                            pst = psM.tile([P, P], bf, tag="mix", name="pst")
                            nc.tensor.transpose(
                                pst[:],
                                ctxn[:, g * DQ + c * P : g * DQ + (c + 1) * P],
                                ident[:],
                            )
                            nc.vector.tensor_copy(
                                ctxt[c][:, g * P : (g + 1) * P], pst[:]
                            )

                    yield t

            def outproj_tasks(qb):
                qs = slice(qb * TQB, (qb + 1) * TQB)
                for n in range(NDC):

                    def t(n=n):
                        pso = psM.tile([P, TQB], f32, tag="mix", name="pso")
                        nc.tensor.matmul(
                            pso[:], woa_sb[:, n * P : (n + 1) * P],
                            ctxt[0][:, qs], start=True, stop=False,
                        )
                        nc.tensor.matmul(
                            pso[:], wob_sb[:, n * P : (n + 1) * P],
                            ctxt[1][:, qs], start=False, stop=True,
                        )
                        osb = obp.tile([P, TQB], f32, tag="osb", name="osb")
                        nc.vector.tensor_copy(osb[:], pso[:])
                        nc.sync.dma_start(ot[n * P : (n + 1) * P, qs], osb[:])

                    yield t

            # ---- software-pipelined schedule ----
            # block qb runs: attention(qb) chunks, interleaved with
            # transposes+outproj of qb-1 and projections of qb+1.
            import contextlib

            _hint = (
                mybir.EngineType.PE,
                mybir.EngineType.Activation,
                mybir.EngineType.DVE,
                mybir.EngineType.Pool,
                mybir.EngineType.SP,
            )
            loop_cm = (
                tc.For_i(0, loop, 1, hint_engines=_hint)
                if loop > 1
                else contextlib.nullcontext()
            )
            with loop_cm:
              for _rep in range(repeat):
                for t in proj_tasks(0):
                    t()
                for qb in range(NQB):
                    aux = []
                    if qb > 0:
                        aux += list(transp_tasks(qb - 1))
                        aux += list(outproj_tasks(qb - 1))
                    if qb + 1 < NQB:
                        aux += list(proj_tasks(qb + 1))
                    nchunks = 4 * (qb + 1)
                    total_chunks = HPC * nchunks
                    psc = psC.tile([P, 3 * 512], f32, tag="ctx", name="psc")
                    # open one accumulation group per ctx bank by zeroing it
                    # (ones x zeros K=1 matmul); the whole-bank write also
                    # forces WAW ordering before every PV accumulate there.
                    for cb in range(3):
                        nc.tensor.matmul(
                            psc[:, cb * 512 : (cb + 1) * 512],
                            ones_sb[:, :],
                            zrow_sb[:, :],
                            start=True,
                            stop=False,
                        )
                    emitted = 0
                    done = 0
                    for h in range(HPC):
                        for ck in range(nchunks):
                            attention_chunk(qb, h, ck, psc)
                            done += 1
                            want = (done * len(aux)) // total_chunks
                            while emitted < want:
                                aux[emitted]()
                                emitted += 1
                    for h in range(HPC):
                        normalize_head(qb, h, psc)
                    while emitted < len(aux):
                        aux[emitted]()
                        emitted += 1
                for t in transp_tasks(NQB - 1):
                    t()
                for t in outproj_tasks(NQB - 1):
                    t()

    nc.compile()
    return nc


def _get_nc():
    if "nc" not in _NC_CACHE:
        _NC_CACHE["nc"] = _build_nc()
    return _NC_CACHE["nc"]


def _in_maps(x, Wq, bq, Wk, bk, Wv, bv, Wo, bo):
    import ml_dtypes

    bf = ml_dtypes.bfloat16
    maps = []
    for c in range(NCORES):
        b, hg = divmod(c, 4)
        cs = slice(hg * DQ, (hg + 1) * DQ)
        bqk_pack = np.stack(
            [
                bq[cs][0:128], bq[cs][128:256],
                bk[cs][0:128], bk[cs][128:256],
            ],
            axis=1,
        ).astype(np.float32)
        maps.append(
            {
                "xT": np.ascontiguousarray(x[b].T).astype(bf),
                "wq": Wq[:, cs].astype(bf),
                "wk": Wk[:, cs].astype(bf),
                "wv": Wv[:, cs].astype(bf),
                "woa": Wo[cs, :][0:128].astype(bf),
                "wob": Wo[cs, :][128:256].astype(bf),
                "bqk": np.ascontiguousarray(bqk_pack),
                "bv1": bv[cs].reshape(1, DQ).astype(bf),
            }
        )
    return maps


def kernel(x, Wq, bq, Wk, bk, Wv, bv, Wo, bo):
    from concourse.bass_utils import run_bass_kernel_spmd

    nc = _get_nc()
    maps = _in_maps(x, Wq, bq, Wk, bk, Wv, bv, Wo, bo)
    res = run_bass_kernel_spmd(nc, maps, list(range(NCORES)))
    out = np.zeros((B, T, D), np.float32)
    for b in range(B):
        acc = res.results[b * 4]["ot"].astype(np.float32)
        for g in range(1, 4):
            acc = acc + res.results[b * 4 + g]["ot"]
        out[b] = acc.T + bo.astype(np.float32)
    return out



# revision 8
# speedup vs baseline: 1686.6538x; 1686.6538x over previous
"""Trainium2 Bass kernel for 16-head causal MHA (B=2, T=4096, D=1024).

Sharding: 8 cores = 2 batches x 4 head-groups (4 heads each).
Each core computes Q/K/V projections for its 256 cols of Wq/Wk/Wv,
streaming causal attention for its 4 heads, and a partial output
projection against its 256 rows of Wo.  Host sums the 4 partials per
batch and adds the output bias.

Device dataflow (all "transposed", T on the free axis):
  xT    [1024, 4096] bf16  (x[b].T)
  qt/kt 2 SBUF tensors [128, 4096] bf16, packing a head PAIR per tile
        (64 partition rows each).
  vsb   per 128-row key-chunk: 4 heads x [64 V cols | ones col] at
        stride 66 ([V|1] trick: PV matmul row 64 = sum of P = l).
  S^T   per (head-pair, key-chunk): TWO K=64 matmuls (row groups 0-63 /
        64-127 run concurrently on the PE sub-arrays) into one
        [128, 1024] PSUM tile.
  P     one exp activation over both heads' scores ([128, 2, 512-co]
        3D AP), PSUM fp32 -> SBUF bf16. Diagonal triangles masked by a
        DVE multiply with a constant lower-tri mask.
  ctx^T accumulated per head in its own PSUM bank as [65, 512]
        (lhsT = [V|1] stationary 65 cols, P moving N=512): rows 0-63 =
        ctx^T, row 64 = l.  Normalised at the end of each pass by
        broadcasting l (gpsimd partition_broadcast), one
        reciprocal_approx_fast, and two DVE multiplies straight into
        ctxt (pre-transposed for the output projection - no PE
        transposes needed).
  out   o^T [1024, 4096] fp32 = Wo_slice^T @ ctxt, host transposes+sums.

Emission is software-pipelined: scores/exp run LAG=2 chunk groups ahead
of PV so the in-order PE queue never blocks on a just-issued exp, and
projection/output/normalise micro-tasks are dripped between chunk
groups to fill the PE while the ScalarE (exp) streams.
"""

import math

import numpy as np

B, T, D = 2, 4096, 1024
H, HD = 16, 64
NCORES = 8
HPC = 4               # heads per core
DQ = HPC * HD         # 256 per-core projection width
P = 128
TQB = 512             # query block
NQB = T // TQB        # 8
NDC = D // P          # 8 contraction chunks for projections
NTC = T // P          # 32 key/T chunks
VST = 4 * 66          # per-T-chunk V stride: 4 heads * (64 data + 1 one + 1 pad)
LAG = 2               # chunk groups between scores/exp and PV emission

_NC_CACHE = {}


def _build_nc(repeat=1, loop=1):
    import concourse.mybir as mybir
    from concourse import bacc
    from concourse.tile import TileContext

    dt = mybir.dt
    bf = dt.bfloat16
    f32 = dt.float32
    AF = mybir.ActivationFunctionType
    ALU = mybir.AluOpType

    nc = bacc.Bacc("TRN2", target_bir_lowering=False, debug=False)

    xT = nc.dram_tensor("xT", [D, T], bf, kind="ExternalInput")
    wq = nc.dram_tensor("wq", [D, DQ], bf, kind="ExternalInput")
    wk = nc.dram_tensor("wk", [D, DQ], bf, kind="ExternalInput")
    wv = nc.dram_tensor("wv", [D, DQ], bf, kind="ExternalInput")
    woa = nc.dram_tensor("woa", [P, D], bf, kind="ExternalInput")
    wob = nc.dram_tensor("wob", [P, D], bf, kind="ExternalInput")
    bqk = nc.dram_tensor("bqk", [P, 4], f32, kind="ExternalInput")
    bv1 = nc.dram_tensor("bv1", [1, DQ], bf, kind="ExternalInput")
    ot = nc.dram_tensor("ot", [D, T], f32, kind="ExternalOutput")

    with TileContext(nc) as tc:
        with (
            tc.tile_pool(name="per", bufs=1) as per,
            tc.tile_pool(name="xp", bufs=2) as xp,
            tc.tile_pool(name="ptp", bufs=4) as ptp,
            tc.tile_pool(name="obp", bufs=3) as obp,
            tc.tile_pool(name="nrm", bufs=2) as nrm,
            tc.tile_pool(name="psS", bufs=2, space="PSUM") as psS,
            tc.tile_pool(name="psC", bufs=1, space="PSUM") as psC,
            tc.tile_pool(name="psM", bufs=2, space="PSUM") as psM,
        ):
            # ---- persistent tensors ----
            qt = [per.tile([P, T], bf, tag=f"qt{i}", name=f"qt{i}") for i in range(2)]
            kt = [per.tile([P, T], bf, tag=f"kt{i}", name=f"kt{i}") for i in range(2)]
            ctxt = [
                per.tile([P, T], bf, tag=f"ctxt{i}", name=f"ctxt{i}") for i in range(2)
            ]
            vsb = per.tile([P, NTC * VST], bf, tag="vsb")
            wq_sb = per.tile([P, NDC * DQ], bf, tag="wq")
            wk_sb = per.tile([P, NDC * DQ], bf, tag="wk")
            wv_sb = per.tile([P, NDC * DQ], bf, tag="wv")
            woa_sb = per.tile([P, D], bf, tag="woa")
            wob_sb = per.tile([P, D], bf, tag="wob")
            bqk_sb = per.tile([P, 4], f32, tag="bqk")
            bv1_sb = per.tile([1, DQ], bf, tag="bv1")
            ones_sb = per.tile([1, P], bf, tag="ones")
            tri_sb = per.tile([P, P], bf, tag="tri")  # lower-tri keep mask

            # ---- loads ----
            for w_sb, w_dr in ((wq_sb, wq), (wk_sb, wk), (wv_sb, wv)):
                nc.sync.dma_start(
                    w_sb[:].rearrange("p (c n) -> p c n", c=NDC),
                    w_dr[:, :].rearrange("(c p) n -> p c n", p=P),
                )
            nc.sync.dma_start(woa_sb[:], woa[:, :])
            nc.sync.dma_start(wob_sb[:], wob[:, :])
            nc.sync.dma_start(bqk_sb[:], bqk[:, :])
            nc.sync.dma_start(bv1_sb[:], bv1[:, :])
            nc.vector.memset(ones_sb[:], 1.0)
            # ones columns for the [V|1] trick (data copies overwrite the rest)
            nc.vector.memset(vsb[:], 1.0)
            # tri[p, f] = 1 where f >= p (query col sees key row), else 0
            # affine_select keeps in_ where the affine cmp is true, else fill
            nc.gpsimd.memset(tri_sb[:], 1.0)
            nc.gpsimd.affine_select(
                out=tri_sb[:], in_=tri_sb[:],
                compare_op=ALU.is_ge, fill=0.0,
                base=0, pattern=[[1, P]], channel_multiplier=-1,
            )

            def load_xblk(tb):
                xblk = xp.tile([P, NDC * TQB], bf, tag="xblk", name="xblk")
                nc.sync.dma_start(
                    xblk[:].rearrange("p (c t) -> p c t", c=NDC),
                    xT[:, :].rearrange("(c p) t -> p c t", p=P)[
                        :, :, tb * TQB : (tb + 1) * TQB
                    ],
                )
                return xblk

            def proj_tasks(tb):
                """Micro-tasks (~4 matmuls each) projecting block tb.
                Order: dma, Q j0, K j0, V t0..t3, Q j1, K j1."""
                qk_tasks = {0: [], 1: []}
                v_tasks = []
                state = {}

                def get_xblk():
                    return state["xblk"]

                def t_dma():
                    state["xblk"] = load_xblk(tb)

                for j in range(2):
                    for w_sb, dst, bcol in ((wq_sb, qt, 0), (wk_sb, kt, 2)):

                        def t_a(w_sb=w_sb, j=j):
                            ps = psM.tile([P, TQB], f32, tag="mix", name="psqk")
                            state["ps"] = ps
                            for d in range(4):
                                nc.tensor.matmul(
                                    ps[:],
                                    w_sb[:, d * DQ + j * P : d * DQ + (j + 1) * P],
                                    get_xblk()[:, d * TQB : (d + 1) * TQB],
                                    start=(d == 0),
                                    stop=False,
                                )

                        def t_b(w_sb=w_sb, dst=dst, bcol=bcol, j=j):
                            ps = state["ps"]
                            for d in range(4, NDC):
                                nc.tensor.matmul(
                                    ps[:],
                                    w_sb[:, d * DQ + j * P : d * DQ + (j + 1) * P],
                                    get_xblk()[:, d * TQB : (d + 1) * TQB],
                                    start=False,
                                    stop=(d == NDC - 1),
                                )
                            nc.vector.tensor_scalar_add(
                                dst[j][:, tb * TQB : (tb + 1) * TQB],
                                ps[:],
                                bqk_sb[:, bcol + j : bcol + j + 1],
                            )

                        qk_tasks[j].append(t_a)
                        qk_tasks[j].append(t_b)

                for t4 in range(4):

                    def v_a(t4=t4):
                        ps = psM.tile([P, TQB], f32, tag="mix", name="psv")
                        state["ps"] = ps
                        for d in range(4):
                            nc.tensor.matmul(
                                ps[:, :DQ],
                                get_xblk()[
                                    :, d * TQB + t4 * P : d * TQB + (t4 + 1) * P
                                ],
                                wv_sb[:, d * DQ : (d + 1) * DQ],
                                start=(d == 0),
                                stop=False,
                            )

                    def v_b(t4=t4):
                        ps = state["ps"]
                        tc_ = tb * 4 + t4
                        for d in range(4, NDC):
                            nc.tensor.matmul(
                                ps[:, :DQ],
                                get_xblk()[
                                    :, d * TQB + t4 * P : d * TQB + (t4 + 1) * P
                                ],
                                wv_sb[:, d * DQ : (d + 1) * DQ],
                                start=False,
                                stop=False,
                            )
                        nc.tensor.matmul(
                            ps[:, :DQ], ones_sb[:, :], bv1_sb[:, :],
                            start=False, stop=True,
                        )
                        # one strided copy places all 4 heads at stride VST/4
                        nc.vector.tensor_copy(
                            vsb[:, tc_ * VST : tc_ * VST + 4 * 66].rearrange(
                                "p (h d) -> p h d", h=HPC
                            )[:, :, 0:HD],
                            ps[:, :DQ].rearrange("p (h d) -> p h d", d=HD),
                        )

                    v_tasks.append(v_a)
                    v_tasks.append(v_b)

                yield t_dma
                yield from qk_tasks[0]
                yield from v_tasks
                yield from qk_tasks[1]

            def outproj_tasks(qb):
                qs = slice(qb * TQB, (qb + 1) * TQB)
                for n in range(NDC):

                    def t(n=n):
                        pso = psM.tile([P, TQB], f32, tag="mix", name="pso")
                        nc.tensor.matmul(
                            pso[:], woa_sb[:, n * P : (n + 1) * P],
                            ctxt[0][:, qs], start=True, stop=False,
                        )
                        nc.tensor.matmul(
                            pso[:], wob_sb[:, n * P : (n + 1) * P],
                            ctxt[1][:, qs], start=False, stop=True,
                        )
                        osb = obp.tile([P, TQB], f32, tag="osb", name="osb")
                        nc.vector.tensor_copy(osb[:], pso[:])
                        nc.sync.dma_start(ot[n * P : (n + 1) * P, qs], osb[:])

                    yield t

            def normalize_tasks(qb, g2, psc):
                """Normalise pass (qb, g2): l rows -> broadcast -> 1/l -> two
                multiplies into ctxt[g2] (even head rows 0-63, odd 64-127)."""
                qs = slice(qb * TQB, (qb + 1) * TQB)

                def t_bcast():
                    lrow = nrm.tile([P, 2 * TQB], f32, tag="lrow", name="lrow")
                    lb = nrm.tile([P, 2 * TQB], f32, tag="lb", name="lb")
                    rb = nrm.tile([P, 2 * TQB], f32, tag="rb", name="rb")
                    # gpsimd cannot read PSUM; stage the l row through SBUF
                    # (cross-partition copy 64 -> 0, validated on HW)
                    nc.vector.tensor_copy(lrow[0:1, :], psc[64:65, :])
                    nc.gpsimd.partition_broadcast(lb[0:64, 0:TQB], lrow[0:1, 0:TQB])
                    nc.gpsimd.partition_broadcast(lb[0:64, TQB:], lrow[0:1, TQB:])
                    nc.vector.reciprocal_approx_fast(rb[0:64, :], lb[0:64, :])
                    return rb

                state = {}

                def t0():
                    state["rb"] = t_bcast()

                def t1():
                    rb = state["rb"]
                    nc.vector.tensor_mul(
                        ctxt[g2][0:64, qs], psc[0:64, 0:TQB], rb[0:64, 0:TQB]
                    )

                def t2():
                    rb = state["rb"]
                    tmp = nrm.tile([P, TQB], bf, tag="ntmp", name="ntmp")
                    nc.vector.tensor_mul(tmp[0:64, :], psc[0:64, TQB:], rb[0:64, TQB:])
                    nc.vector.tensor_copy(ctxt[g2][64:128, qs], tmp[0:64, :])

                return [t0, t1, t2]

            def attention_pass(qb, g2, aux):
                """Causal attention for heads (2*g2, 2*g2+1) over query block
                qb. aux tasks are dripped between chunk groups."""
                nchunks = 4 * (qb + 1)
                q0 = qb * TQB
                psc = psC.tile([P, 2 * TQB], f32, tag="ctx", name="psc")
                pts = {}

                def scores_exp(ck):
                    tk0 = ck * P
                    co = max(0, tk0 - q0)
                    pss = psS.tile([P, 2 * TQB], f32, tag="scores", name="pss")
                    pt = ptp.tile([P, 2 * TQB], bf, tag="pt", name="pt")
                    for hh in range(2):
                        nc.tensor.matmul(
                            pss[:, hh * TQB + co : (hh + 1) * TQB],
                            kt[g2][hh * 64 : (hh + 1) * 64, tk0 : tk0 + P],
                            qt[g2][hh * 64 : (hh + 1) * 64, q0 + co : q0 + TQB],
                            start=True,
                            stop=True,
                        )
                    nc.scalar.activation(
                        pt[:].rearrange("p (h q) -> p h q", h=2)[:, :, co:TQB],
                        pss[:].rearrange("p (h q) -> p h q", h=2)[:, :, co:TQB],
                        AF.Exp,
                        scale=1.0 / math.sqrt(HD),
                    )
                    if tk0 >= q0:
                        # diagonal chunk: zero the upper-left triangle via a
                        # DVE multiply with the lower-tri keep mask
                        for hh in range(2):
                            tgt = pt[:, hh * TQB + co : hh * TQB + co + P]
                            nc.vector.tensor_mul(tgt, tgt, tri_sb[:])
                    pts[ck] = (pt, co)

                def pv(ck):
                    pt, co = pts.pop(ck)
                    tc_ = ck
                    for hh in range(2):
                        h = 2 * g2 + hh
                        nc.tensor.matmul(
                            psc[0:65, hh * TQB + co : (hh + 1) * TQB],
                            vsb[:, tc_ * VST + h * 66 : tc_ * VST + h * 66 + 65],
                            pt[:, hh * TQB + co : (hh + 1) * TQB],
                            start=(ck == 0),
                            stop=(ck == nchunks - 1),
                        )

                emitted = 0
                done = 0
                total = nchunks + 1
                for ck in range(nchunks):
                    scores_exp(ck)
                    if ck >= LAG:
                        pv(ck - LAG)
                    done += 1
                    want = (done * len(aux)) // total
                    while emitted < want:
                        aux[emitted]()
                        emitted += 1
                for ck in range(max(0, nchunks - LAG), nchunks):
                    pv(ck)
                while emitted < len(aux):
                    aux[emitted]()
                    emitted += 1
                return psc

            # ---- software-pipelined schedule ----
            import contextlib

            _hint = (
                mybir.EngineType.PE,
                mybir.EngineType.Activation,
                mybir.EngineType.DVE,
                mybir.EngineType.Pool,
                mybir.EngineType.SP,
            )
            loop_cm = (
                tc.For_i(0, loop, 1, hint_engines=_hint)
                if loop > 1
                else contextlib.nullcontext()
            )
            with loop_cm:
              for _rep in range(repeat):
                for t in proj_tasks(0):
                    t()
                prev_norm = None  # normalize tasks of the previous pass
                for qb in range(NQB):
                    for g2 in range(2):
                        aux = []
                        if prev_norm is not None:
                            aux += prev_norm
                        if g2 == 0:
                            if qb + 1 < NQB:
                                pj = list(proj_tasks(qb + 1))
                                # dma + Q j0 + K j0 first half
                                aux += pj[0:5]
                                rest = pj[5:]
                            else:
                                rest = []
                            if qb > 0:
                                aux += list(outproj_tasks(qb - 1))
                            state_rest = rest
                        else:
                            aux += state_rest
                        psc = attention_pass(qb, g2, aux)
                        prev_norm = normalize_tasks(qb, g2, psc)
                for t in prev_norm:
                    t()
                for t in outproj_tasks(NQB - 1):
                    t()

    nc.compile()
    return nc


def _get_nc():
    if "nc" not in _NC_CACHE:
        _NC_CACHE["nc"] = _build_nc()
    return _NC_CACHE["nc"]


def _in_maps(x, Wq, bq, Wk, bk, Wv, bv, Wo, bo):
    import ml_dtypes

    bf = ml_dtypes.bfloat16
    maps = []
    for c in range(NCORES):
        b, hg = divmod(c, 4)
        cs = slice(hg * DQ, (hg + 1) * DQ)
        bqk_pack = np.stack(
            [
                bq[cs][0:128], bq[cs][128:256],
                bk[cs][0:128], bk[cs][128:256],
            ],
            axis=1,
        ).astype(np.float32)
        maps.append(
            {
                "xT": np.ascontiguousarray(x[b].T).astype(bf),
                "wq": Wq[:, cs].astype(bf),
                "wk": Wk[:, cs].astype(bf),
                "wv": Wv[:, cs].astype(bf),
                "woa": Wo[cs, :][0:128].astype(bf),
                "wob": Wo[cs, :][128:256].astype(bf),
                "bqk": np.ascontiguousarray(bqk_pack),
                "bv1": bv[cs].reshape(1, DQ).astype(bf),
            }
        )
    return maps


def kernel(x, Wq, bq, Wk, bk, Wv, bv, Wo, bo):
    from concourse.bass_utils import run_bass_kernel_spmd

    nc = _get_nc()
    maps = _in_maps(x, Wq, bq, Wk, bk, Wv, bv, Wo, bo)
    res = run_bass_kernel_spmd(nc, maps, list(range(NCORES)))
    out = np.zeros((B, T, D), np.float32)
    for b in range(B):
        acc = res.results[b * 4]["ot"].astype(np.float32)
        for g in range(1, 4):
            acc = acc + res.results[b * 4 + g]["ot"]
        out[b] = acc.T + bo.astype(np.float32)
    return out


# revision 16
# speedup vs baseline: 1869.8557x; 1.1086x over previous
"""Trainium2 Bass kernel for 16-head causal MHA (B=2, T=4096, D=1024).

Sharding: 8 cores = 2 batches x 4 head-groups (4 heads each).
Each core computes Q/K/V projections for its 256 cols of Wq/Wk/Wv,
streaming causal attention for its 4 heads, and a partial output
projection against its 256 rows of Wo.  Host sums the 4 partials per
batch and adds the output bias.

Device dataflow (all "transposed", T on the free axis):
  xT    [1024, 4096] bf16  (x[b].T)
  qt/kt 2 SBUF tensors [128, 4096] bf16, packing a head PAIR per tile
        (64 partition rows each).
  vsb   per 128-row key-chunk: 4 heads x [64 V cols | ones col] at
        stride 66 ([V|1] trick: PV matmul row 64 = sum of P = l).
  S^T   per (head-pair, key-chunk): TWO K=64 matmuls (row groups 0-63 /
        64-127 run concurrently on the PE sub-arrays) into one
        [128, 1024] PSUM tile.
  P     one exp activation over both heads' scores ([128, 2, 512-co]
        3D AP), PSUM fp32 -> SBUF bf16. Diagonal triangles masked by a
        DVE multiply with a constant lower-tri mask.
  ctx^T accumulated per head in its own PSUM bank as [65, 512]
        (lhsT = [V|1] stationary 65 cols, P moving N=512): rows 0-63 =
        ctx^T, row 64 = l.  Normalised at the end of each pass by
        broadcasting l (gpsimd partition_broadcast), one
        reciprocal_approx_fast, and two DVE multiplies straight into
        ctxt (pre-transposed for the output projection - no PE
        transposes needed).
  out   o^T [1024, 4096] fp32 = Wo_slice^T @ ctxt, host transposes+sums.

Emission is software-pipelined: scores/exp run LAG=2 chunk groups ahead
of PV so the in-order PE queue never blocks on a just-issued exp, and
projection/output/normalise micro-tasks are dripped between chunk
groups to fill the PE while the ScalarE (exp) streams.
"""

import math

import numpy as np

B, T, D = 2, 4096, 1024
H, HD = 16, 64
NCORES = 8
HPC = 4               # heads per core
DQ = HPC * HD         # 256 per-core projection width
P = 128
TQB = 512             # query block
NQB = T // TQB        # 8
NDC = D // P          # 8 contraction chunks for projections
NTC = T // P          # 32 key/T chunks
VST = 4 * 66          # per-T-chunk V stride: 4 heads * (64 data + 1 one + 1 pad)
LAG = 2               # chunk groups between scores/exp and PV emission
DVE_EXP_MOD = 4       # offload every Nth chunk group's exp to DVE (0 = off)

_NC_CACHE = {}


def _build_nc(repeat=1, loop=1):
    import concourse.mybir as mybir
    from concourse import bacc
    from concourse.tile import TileContext

    dt = mybir.dt
    bf = dt.bfloat16
    f32 = dt.float32
    i16 = dt.int16
    AF = mybir.ActivationFunctionType
    ALU = mybir.AluOpType
    # DVE fast-exp: bf16(int16(s*FE_A + FE_B)) ~= exp(s/8) to ~3% rel
    FE_A = 128.0 / math.log(2.0) / 8.0
    FE_B = 127.0 * 128.0 - 5.5 + 0.5

    nc = bacc.Bacc("TRN2", target_bir_lowering=False, debug=False)

    xT = nc.dram_tensor("xT", [D, T], bf, kind="ExternalInput")
    wq = nc.dram_tensor("wq", [D, DQ], bf, kind="ExternalInput")
    wk = nc.dram_tensor("wk", [D, DQ], bf, kind="ExternalInput")
    wv = nc.dram_tensor("wv", [D, DQ], bf, kind="ExternalInput")
    woa = nc.dram_tensor("woa", [P, D], bf, kind="ExternalInput")
    wob = nc.dram_tensor("wob", [P, D], bf, kind="ExternalInput")
    bqk = nc.dram_tensor("bqk", [P, 4], f32, kind="ExternalInput")
    bv1 = nc.dram_tensor("bv1", [1, DQ], bf, kind="ExternalInput")
    ot = nc.dram_tensor("ot", [D, T], f32, kind="ExternalOutput")

    with TileContext(nc) as tc:
        with (
            tc.tile_pool(name="per", bufs=1) as per,
            tc.tile_pool(name="xp", bufs=2) as xp,
            tc.tile_pool(name="ptp", bufs=4) as ptp,
            tc.tile_pool(name="obp", bufs=3) as obp,
            tc.tile_pool(name="nrm", bufs=2) as nrm,
            tc.tile_pool(name="psS", bufs=2, space="PSUM") as psS,
            tc.tile_pool(name="psC", bufs=1, space="PSUM") as psC,
            tc.tile_pool(name="psM", bufs=2, space="PSUM") as psM,
        ):
            # ---- persistent tensors ----
            qt = [per.tile([P, T], bf, tag=f"qt{i}", name=f"qt{i}") for i in range(2)]
            kt = [per.tile([P, T], bf, tag=f"kt{i}", name=f"kt{i}") for i in range(2)]
            ctxt = [
                per.tile([P, T], bf, tag=f"ctxt{i}", name=f"ctxt{i}") for i in range(2)
            ]
            vsb = per.tile([P, NTC * VST], bf, tag="vsb")
            wq_sb = per.tile([P, NDC * DQ], bf, tag="wq")
            wk_sb = per.tile([P, NDC * DQ], bf, tag="wk")
            wv_sb = per.tile([P, NDC * DQ], bf, tag="wv")
            woa_sb = per.tile([P, D], bf, tag="woa")
            wob_sb = per.tile([P, D], bf, tag="wob")
            bqk_sb = per.tile([P, 4], f32, tag="bqk")
            bv1_sb = per.tile([1, DQ], bf, tag="bv1")
            ones_sb = per.tile([1, P], bf, tag="ones")
            tri_sb = per.tile([P, P], bf, tag="tri")  # lower-tri keep mask

            # ---- loads ----
            for w_sb, w_dr in ((wq_sb, wq), (wk_sb, wk), (wv_sb, wv)):
                nc.sync.dma_start(
                    w_sb[:].rearrange("p (c n) -> p c n", c=NDC),
                    w_dr[:, :].rearrange("(c p) n -> p c n", p=P),
                )
            nc.sync.dma_start(woa_sb[:], woa[:, :])
            nc.sync.dma_start(wob_sb[:], wob[:, :])
            nc.sync.dma_start(bqk_sb[:], bqk[:, :])
            nc.sync.dma_start(bv1_sb[:], bv1[:, :])
            nc.vector.memset(ones_sb[:], 1.0)
            # ones columns for the [V|1] trick (data copies overwrite the rest)
            nc.vector.memset(vsb[:], 1.0)
            # tri[p, f] = 1 where f >= p (query col sees key row), else 0
            # affine_select keeps in_ where the affine cmp is true, else fill
            nc.gpsimd.memset(tri_sb[:], 1.0)
            nc.gpsimd.affine_select(
                out=tri_sb[:], in_=tri_sb[:],
                compare_op=ALU.is_ge, fill=0.0,
                base=0, pattern=[[1, P]], channel_multiplier=-1,
            )

            gctr = {"n": 0}

            def load_xblk(tb):
                xblk = xp.tile([P, NDC * TQB], bf, tag="xblk", name="xblk")
                nc.sync.dma_start(
                    xblk[:].rearrange("p (c t) -> p c t", c=NDC),
                    xT[:, :].rearrange("(c p) t -> p c t", p=P)[
                        :, :, tb * TQB : (tb + 1) * TQB
                    ],
                )
                return xblk

            def proj_tasks(tb):
                """Micro-tasks (~4 matmuls each) projecting block tb.
                Order: dma, Q j0, K j0, V t0..t3, Q j1, K j1."""
                qk_tasks = {0: [], 1: []}
                v_tasks = []
                state = {}

                def get_xblk():
                    return state["xblk"]

                def t_dma():
                    state["xblk"] = load_xblk(tb)

                for j in range(2):
                    for w_sb, dst, bcol in ((wq_sb, qt, 0), (wk_sb, kt, 2)):

                        def t_a(w_sb=w_sb, j=j):
                            ps = psM.tile([P, TQB], f32, tag="mix", name="psqk")
                            state["ps"] = ps
                            for d in range(4):
                                nc.tensor.matmul(
                                    ps[:],
                                    w_sb[:, d * DQ + j * P : d * DQ + (j + 1) * P],
                                    get_xblk()[:, d * TQB : (d + 1) * TQB],
                                    start=(d == 0),
                                    stop=False,
                                )

                        def t_b(w_sb=w_sb, dst=dst, bcol=bcol, j=j):
                            ps = state["ps"]
                            for d in range(4, NDC):
                                nc.tensor.matmul(
                                    ps[:],
                                    w_sb[:, d * DQ + j * P : d * DQ + (j + 1) * P],
                                    get_xblk()[:, d * TQB : (d + 1) * TQB],
                                    start=False,
                                    stop=(d == NDC - 1),
                                )
                            nc.vector.tensor_scalar_add(
                                dst[j][:, tb * TQB : (tb + 1) * TQB],
                                ps[:],
                                bqk_sb[:, bcol + j : bcol + j + 1],
                            )

                        qk_tasks[j].append(t_a)
                        qk_tasks[j].append(t_b)

                for t4 in range(4):

                    def v_a(t4=t4):
                        ps = psM.tile([P, TQB], f32, tag="mix", name="psv")
                        state["ps"] = ps
                        for d in range(4):
                            nc.tensor.matmul(
                                ps[:, :DQ],
                                get_xblk()[
                                    :, d * TQB + t4 * P : d * TQB + (t4 + 1) * P
                                ],
                                wv_sb[:, d * DQ : (d + 1) * DQ],
                                start=(d == 0),
                                stop=False,
                            )

                    def v_b(t4=t4):
                        ps = state["ps"]
                        tc_ = tb * 4 + t4
                        for d in range(4, NDC):
                            nc.tensor.matmul(
                                ps[:, :DQ],
                                get_xblk()[
                                    :, d * TQB + t4 * P : d * TQB + (t4 + 1) * P
                                ],
                                wv_sb[:, d * DQ : (d + 1) * DQ],
                                start=False,
                                stop=False,
                            )
                        nc.tensor.matmul(
                            ps[:, :DQ], ones_sb[:, :], bv1_sb[:, :],
                            start=False, stop=True,
                        )
                        # one strided copy places all 4 heads at stride VST/4
                        nc.vector.tensor_copy(
                            vsb[:, tc_ * VST : tc_ * VST + 4 * 66].rearrange(
                                "p (h d) -> p h d", h=HPC
                            )[:, :, 0:HD],
                            ps[:, :DQ].rearrange("p (h d) -> p h d", d=HD),
                        )

                    v_tasks.append(v_a)
                    v_tasks.append(v_b)

                yield t_dma
                yield from qk_tasks[0]
                yield from v_tasks
                yield from qk_tasks[1]

            def outproj_tasks(qb):
                qs = slice(qb * TQB, (qb + 1) * TQB)
                for n in range(NDC):

                    def t(n=n):
                        pso = psM.tile([P, TQB], f32, tag="mix", name="pso")
                        nc.tensor.matmul(
                            pso[:], woa_sb[:, n * P : (n + 1) * P],
                            ctxt[0][:, qs], start=True, stop=False,
                        )
                        nc.tensor.matmul(
                            pso[:], wob_sb[:, n * P : (n + 1) * P],
                            ctxt[1][:, qs], start=False, stop=True,
                        )
                        osb = obp.tile([P, TQB], f32, tag="osb", name="osb")
                        nc.vector.tensor_copy(osb[:], pso[:])
                        nc.sync.dma_start(ot[n * P : (n + 1) * P, qs], osb[:])

                    yield t

            def normalize_tasks(qb, g2, psc):
                """Normalise pass (qb, g2): l rows -> broadcast -> 1/l -> two
                multiplies into ctxt[g2] (even head rows 0-63, odd 64-127)."""
                qs = slice(qb * TQB, (qb + 1) * TQB)

                def t_bcast():
                    lrow = nrm.tile([P, 2 * TQB], f32, tag="lrow", name="lrow")
                    lb = nrm.tile([P, 2 * TQB], f32, tag="lb", name="lb")
                    rb = nrm.tile([P, 2 * TQB], f32, tag="rb", name="rb")
                    # gpsimd cannot read PSUM; stage the l row through SBUF
                    # (cross-partition copy 64 -> 0, validated on HW)
                    nc.vector.tensor_copy(lrow[0:1, :], psc[64:65, :])
                    nc.gpsimd.partition_broadcast(lb[0:64, 0:TQB], lrow[0:1, 0:TQB])
                    nc.gpsimd.partition_broadcast(lb[0:64, TQB:], lrow[0:1, TQB:])
                    nc.vector.reciprocal_approx_fast(rb[0:64, :], lb[0:64, :])
                    return rb

                state = {}

                def t0():
                    state["rb"] = t_bcast()

                def t1():
                    rb = state["rb"]
                    nc.vector.tensor_mul(
                        ctxt[g2][0:64, qs], psc[0:64, 0:TQB], rb[0:64, 0:TQB]
                    )

                def t2():
                    rb = state["rb"]
                    tmp = nrm.tile([P, TQB], bf, tag="ntmp", name="ntmp")
                    nc.vector.tensor_mul(tmp[0:64, :], psc[0:64, TQB:], rb[0:64, TQB:])
                    nc.vector.tensor_copy(ctxt[g2][64:128, qs], tmp[0:64, :])

                return [t0, t1, t2]

            def attention_pass(qb, g2, aux):
                """Causal attention for heads (2*g2, 2*g2+1) over query block
                qb. aux tasks are dripped between chunk groups."""
                nchunks = 4 * (qb + 1)
                q0 = qb * TQB
                psc = psC.tile([P, 2 * TQB], f32, tag="ctx", name="psc")
                pts = {}

                def scores_exp(ck):
                    tk0 = ck * P
                    co = max(0, tk0 - q0)
                    pss = psS.tile([P, 2 * TQB], f32, tag="scores", name="pss")
                    pt = ptp.tile([P, 2 * TQB], bf, tag="pt", name="pt")
                    for hh in range(2):
                        nc.tensor.matmul(
                            pss[:, hh * TQB + co : (hh + 1) * TQB],
                            kt[g2][hh * 64 : (hh + 1) * 64, tk0 : tk0 + P],
                            qt[g2][hh * 64 : (hh + 1) * 64, q0 + co : q0 + TQB],
                            start=True,
                            stop=True,
                        )
                    src3 = pss[:].rearrange("p (h q) -> p h q", h=2)[:, :, co:TQB]
                    gctr["n"] += 1
                    if DVE_EXP_MOD and gctr["n"] % DVE_EXP_MOD == 0:
                        nc.vector.tensor_scalar(
                            pt[:].bitcast(i16).rearrange("p (h q) -> p h q", h=2)[
                                :, :, co:TQB
                            ],
                            src3,
                            FE_A,
                            FE_B,
                            op0=ALU.mult,
                            op1=ALU.add,
                        )
                    else:
                        nc.scalar.activation(
                            pt[:].rearrange("p (h q) -> p h q", h=2)[:, :, co:TQB],
                            src3,
                            AF.Exp,
                            scale=1.0 / math.sqrt(HD),
                        )
                    if tk0 >= q0:
                        # diagonal chunk: zero the upper-left triangle via a
                        # DVE multiply with the lower-tri keep mask
                        for hh in range(2):
                            tgt = pt[:, hh * TQB + co : hh * TQB + co + P]
                            nc.vector.tensor_mul(tgt, tgt, tri_sb[:])
                    pts[ck] = (pt, co)

                def pv(ck):
                    pt, co = pts.pop(ck)
                    tc_ = ck
                    for hh in range(2):
                        h = 2 * g2 + hh
                        nc.tensor.matmul(
                            psc[0:65, hh * TQB + co : (hh + 1) * TQB],
                            vsb[:, tc_ * VST + h * 66 : tc_ * VST + h * 66 + 65],
                            pt[:, hh * TQB + co : (hh + 1) * TQB],
                            start=(ck == 0),
                            stop=(ck == nchunks - 1),
                        )

                emitted = 0
                done = 0
                total = nchunks + 1
                for ck in range(nchunks):
                    scores_exp(ck)
                    if ck >= LAG:
                        pv(ck - LAG)
                    done += 1
                    want = (done * len(aux)) // total
                    while emitted < want:
                        aux[emitted]()
                        emitted += 1
                for ck in range(max(0, nchunks - LAG), nchunks):
                    pv(ck)
                while emitted < len(aux):
                    aux[emitted]()
                    emitted += 1
                return psc

            # ---- software-pipelined schedule ----
            import contextlib

            _hint = (
                mybir.EngineType.PE,
                mybir.EngineType.Activation,
                mybir.EngineType.DVE,
                mybir.EngineType.Pool,
                mybir.EngineType.SP,
            )
            loop_cm = (
                tc.For_i(0, loop, 1, hint_engines=_hint)
                if loop > 1
                else contextlib.nullcontext()
            )
            with loop_cm:
              for _rep in range(repeat):
                for t in proj_tasks(0):
                    t()
                prev_norm = None  # normalize tasks of the previous pass
                for qb in range(NQB):
                    for g2 in range(2):
                        aux = []
                        if prev_norm is not None:
                            aux += prev_norm
                        if g2 == 0:
                            if qb + 1 < NQB:
                                pj = list(proj_tasks(qb + 1))
                                # dma + Q j0 + K j0 first half
                                aux += pj[0:5]
                                rest = pj[5:]
                            else:
                                rest = []
                            if qb > 0:
                                aux += list(outproj_tasks(qb - 1))
                            state_rest = rest
                        else:
                            aux += state_rest
                        psc = attention_pass(qb, g2, aux)
                        prev_norm = normalize_tasks(qb, g2, psc)
                for t in prev_norm:
                    t()
                for t in outproj_tasks(NQB - 1):
                    t()

    nc.compile()
    return nc


def _get_nc():
    if "nc" not in _NC_CACHE:
        _NC_CACHE["nc"] = _build_nc()
    return _NC_CACHE["nc"]


def _in_maps(x, Wq, bq, Wk, bk, Wv, bv, Wo, bo):
    import ml_dtypes

    bf = ml_dtypes.bfloat16
    maps = []
    for c in range(NCORES):
        b, hg = divmod(c, 4)
        cs = slice(hg * DQ, (hg + 1) * DQ)
        bqk_pack = np.stack(
            [
                bq[cs][0:128], bq[cs][128:256],
                bk[cs][0:128], bk[cs][128:256],
            ],
            axis=1,
        ).astype(np.float32)
        maps.append(
            {
                "xT": np.ascontiguousarray(x[b].T).astype(bf),
                "wq": Wq[:, cs].astype(bf),
                "wk": Wk[:, cs].astype(bf),
                "wv": Wv[:, cs].astype(bf),
                "woa": Wo[cs, :][0:128].astype(bf),
                "wob": Wo[cs, :][128:256].astype(bf),
                "bqk": np.ascontiguousarray(bqk_pack),
                "bv1": bv[cs].reshape(1, DQ).astype(bf),
            }
        )
    return maps


def kernel(x, Wq, bq, Wk, bk, Wv, bv, Wo, bo):
    from concourse.bass_utils import run_bass_kernel_spmd

    nc = _get_nc()
    maps = _in_maps(x, Wq, bq, Wk, bk, Wv, bv, Wo, bo)
    res = run_bass_kernel_spmd(nc, maps, list(range(NCORES)))
    out = np.zeros((B, T, D), np.float32)
    for b in range(B):
        acc = res.results[b * 4]["ot"].astype(np.float32)
        for g in range(1, 4):
            acc = acc + res.results[b * 4 + g]["ot"]
        out[b] = acc.T + bo.astype(np.float32)
    return out


# revision 21
# speedup vs baseline: 2113.3987x; 1.1302x over previous
"""Trainium2 Bass kernel for 16-head causal MHA (B=2, T=4096, D=1024).

Sharding: 8 cores = 2 batches x 4 head-groups (4 heads each).
Each core computes Q/K/V projections for its 256 cols of Wq/Wk/Wv,
streaming causal attention for its 4 heads, and a partial output
projection against its 256 rows of Wo.  Host sums the 4 partials per
batch and adds the output bias.

Device dataflow (all "transposed", T on the free axis):
  xT    [1024, 4096] bf16  (x[b].T)
  qt/kt 2 SBUF tensors [128, 4096] bf16, packing a head PAIR per tile
        (64 partition rows each).
  vsb   per 128-row key-chunk: 4 heads x [64 V cols | ones col] at
        stride 66 ([V|1] trick: PV matmul row 64 = sum of P = l).
  S^T   per (head-pair, key-chunk): TWO K=64 matmuls (row groups 0-63 /
        64-127 run concurrently on the PE sub-arrays) into one
        [128, 1024] PSUM tile.
  P     one exp activation over both heads' scores ([128, 2, 512-co]
        3D AP), PSUM fp32 -> SBUF bf16. Diagonal triangles masked by a
        DVE multiply with a constant lower-tri mask.
  ctx^T accumulated per head in its own PSUM bank as [65, 512]
        (lhsT = [V|1] stationary 65 cols, P moving N=512): rows 0-63 =
        ctx^T, row 64 = l.  Normalised at the end of each pass by
        broadcasting l (gpsimd partition_broadcast), one
        reciprocal_approx_fast, and two DVE multiplies straight into
        ctxt (pre-transposed for the output projection - no PE
        transposes needed).
  out   o^T [1024, 4096] fp32 = Wo_slice^T @ ctxt, host transposes+sums.

Emission is software-pipelined: scores/exp run LAG=2 chunk groups ahead
of PV so the in-order PE queue never blocks on a just-issued exp, and
projection/output/normalise micro-tasks are dripped between chunk
groups to fill the PE while the ScalarE (exp) streams.
"""

import math

import numpy as np

B, T, D = 2, 4096, 1024
H, HD = 16, 64
NCORES = 8
HPC = 4               # heads per core
DQ = HPC * HD         # 256 per-core projection width
P = 128
TQB = 512             # query block
NQB = T // TQB        # 8
NDC = D // P          # 8 contraction chunks for projections
NTC = T // P          # 32 key/T chunks
VST = 4 * 66          # per-T-chunk V stride: 4 heads * (64 data + 1 one + 1 pad)
LAG = 2               # chunk groups between scores/exp and PV emission
DVE_EXP_MOD = 4       # offload every Nth chunk group's exp to DVE (0 = off)

_NC_CACHE = {}


def _build_nc(repeat=1, loop=1):
    import concourse.mybir as mybir
    from concourse import bacc
    from concourse.tile import TileContext

    dt = mybir.dt
    bf = dt.bfloat16
    f32 = dt.float32
    i16 = dt.int16
    AF = mybir.ActivationFunctionType
    ALU = mybir.AluOpType
    # DVE fast-exp: bf16(int16(s*FE_A + FE_B)) ~= exp(s/8) to ~3% rel
    FE_A = 128.0 / math.log(2.0) / 8.0
    FE_B = 127.0 * 128.0 - 5.5 + 0.5

    nc = bacc.Bacc("TRN2", target_bir_lowering=False, debug=False)

    xT = nc.dram_tensor("xT", [D, T], bf, kind="ExternalInput")
    wq = nc.dram_tensor("wq", [D, DQ], bf, kind="ExternalInput")
    wk = nc.dram_tensor("wk", [D, DQ], bf, kind="ExternalInput")
    wv = nc.dram_tensor("wv", [D, DQ], bf, kind="ExternalInput")
    woa = nc.dram_tensor("woa", [P, D], bf, kind="ExternalInput")
    wob = nc.dram_tensor("wob", [P, D], bf, kind="ExternalInput")
    bqk = nc.dram_tensor("bqk", [P, 4], f32, kind="ExternalInput")
    bv1 = nc.dram_tensor("bv1", [1, DQ], bf, kind="ExternalInput")
    ot = nc.dram_tensor("ot", [D, T], f32, kind="ExternalOutput")

    with TileContext(nc) as tc:
        with (
            tc.tile_pool(name="per", bufs=1) as per,
            tc.tile_pool(name="xp", bufs=2) as xp,
            tc.tile_pool(name="ptp", bufs=4) as ptp,
            tc.tile_pool(name="obp", bufs=3) as obp,
            tc.tile_pool(name="nrm", bufs=2) as nrm,
            tc.tile_pool(name="psS", bufs=2, space="PSUM") as psS,
            tc.tile_pool(name="psC", bufs=1, space="PSUM") as psC,
            tc.tile_pool(name="psM", bufs=2, space="PSUM") as psM,
        ):
            # ---- persistent tensors ----
            qt = [per.tile([P, T], bf, tag=f"qt{i}", name=f"qt{i}") for i in range(2)]
            kt = [per.tile([P, T], bf, tag=f"kt{i}", name=f"kt{i}") for i in range(2)]
            ctxt = [
                per.tile([P, T], bf, tag=f"ctxt{i}", name=f"ctxt{i}") for i in range(2)
            ]
            vsb = per.tile([P, NTC * VST], bf, tag="vsb")
            wq_sb = per.tile([P, NDC * DQ], bf, tag="wq")
            wk_sb = per.tile([P, NDC * DQ], bf, tag="wk")
            wv_sb = per.tile([P, NDC * DQ], bf, tag="wv")
            woa_sb = per.tile([P, D], bf, tag="woa")
            wob_sb = per.tile([P, D], bf, tag="wob")
            bqk_sb = per.tile([P, 4], f32, tag="bqk")
            bv1_sb = per.tile([1, DQ], bf, tag="bv1")
            ones_sb = per.tile([1, P], bf, tag="ones")
            tri_sb = per.tile([P, P], bf, tag="tri")  # lower-tri keep mask

            # ---- loads ----
            # split the first-needed weights in half so the first Q-proj
            # matmul (contraction chunks 0-3) starts ~halfway into the load
            for w_sb, w_dr in ((wq_sb, wq), (wk_sb, wk), (wv_sb, wv)):
                for h0, h1 in ((0, 4), (4, NDC)):
                    nc.sync.dma_start(
                        w_sb[:].rearrange("p (c n) -> p c n", c=NDC)[:, h0:h1],
                        w_dr[:, :].rearrange("(c p) n -> p c n", p=P)[:, h0:h1],
                    )
            nc.sync.dma_start(woa_sb[:], woa[:, :])
            nc.sync.dma_start(wob_sb[:], wob[:, :])
            nc.sync.dma_start(bqk_sb[:], bqk[:, :])
            nc.sync.dma_start(bv1_sb[:], bv1[:, :])
            nc.vector.memset(ones_sb[:], 1.0)
            # ones columns for the [V|1] trick (data copies overwrite the rest)
            nc.vector.memset(vsb[:], 1.0)
            # tri[p, f] = 1 where f >= p (query col sees key row), else 0
            # affine_select keeps in_ where the affine cmp is true, else fill
            nc.gpsimd.memset(tri_sb[:], 1.0)
            nc.gpsimd.affine_select(
                out=tri_sb[:], in_=tri_sb[:],
                compare_op=ALU.is_ge, fill=0.0,
                base=0, pattern=[[1, P]], channel_multiplier=-1,
            )

            gctr = {"n": 0}

            def load_xblk(tb):
                xblk = xp.tile([P, NDC * TQB], bf, tag="xblk", name="xblk")
                nc.sync.dma_start(
                    xblk[:].rearrange("p (c t) -> p c t", c=NDC),
                    xT[:, :].rearrange("(c p) t -> p c t", p=P)[
                        :, :, tb * TQB : (tb + 1) * TQB
                    ],
                )
                return xblk

            xblks = {}

            def proj_tasks(tb):
                """Micro-tasks (~4 matmuls each) projecting block tb.
                Order: Q j0, K j0, V t0..t3, Q j1, K j1.  The xblk DMA is
                issued separately (a full pass earlier) via dma_task."""
                qk_tasks = {0: [], 1: []}
                v_tasks = []
                state = {}

                def get_xblk():
                    return xblks[tb]

                for j in range(2):
                    for w_sb, dst, bcol in ((wq_sb, qt, 0), (wk_sb, kt, 2)):

                        def t_a(w_sb=w_sb, j=j):
                            ps = psM.tile([P, TQB], f32, tag="mix", name="psqk")
                            state["ps"] = ps
                            for d in range(4):
                                nc.tensor.matmul(
                                    ps[:],
                                    w_sb[:, d * DQ + j * P : d * DQ + (j + 1) * P],
                                    get_xblk()[:, d * TQB : (d + 1) * TQB],
                                    start=(d == 0),
                                    stop=False,
                                )

                        def t_b(w_sb=w_sb, dst=dst, bcol=bcol, j=j):
                            ps = state["ps"]
                            for d in range(4, NDC):
                                nc.tensor.matmul(
                                    ps[:],
                                    w_sb[:, d * DQ + j * P : d * DQ + (j + 1) * P],
                                    get_xblk()[:, d * TQB : (d + 1) * TQB],
                                    start=False,
                                    stop=(d == NDC - 1),
                                )
                            nc.vector.tensor_scalar_add(
                                dst[j][:, tb * TQB : (tb + 1) * TQB],
                                ps[:],
                                bqk_sb[:, bcol + j : bcol + j + 1],
                            )

                        qk_tasks[j].append(t_a)
                        qk_tasks[j].append(t_b)

                for t4 in range(4):

                    def v_a(t4=t4):
                        ps = psM.tile([P, TQB], f32, tag="mix", name="psv")
                        state["ps"] = ps
                        for d in range(4):
                            nc.tensor.matmul(
                                ps[:, :DQ],
                                get_xblk()[
                                    :, d * TQB + t4 * P : d * TQB + (t4 + 1) * P
                                ],
                                wv_sb[:, d * DQ : (d + 1) * DQ],
                                start=(d == 0),
                                stop=False,
                            )

                    def v_b(t4=t4):
                        ps = state["ps"]
                        tc_ = tb * 4 + t4
                        for d in range(4, NDC):
                            nc.tensor.matmul(
                                ps[:, :DQ],
                                get_xblk()[
                                    :, d * TQB + t4 * P : d * TQB + (t4 + 1) * P
                                ],
                                wv_sb[:, d * DQ : (d + 1) * DQ],
                                start=False,
                                stop=False,
                            )
                        nc.tensor.matmul(
                            ps[:, :DQ], ones_sb[:, :], bv1_sb[:, :],
                            start=False, stop=True,
                        )
                        # one strided copy places all 4 heads at stride VST/4
                        nc.vector.tensor_copy(
                            vsb[:, tc_ * VST : tc_ * VST + 4 * 66].rearrange(
                                "p (h d) -> p h d", h=HPC
                            )[:, :, 0:HD],
                            ps[:, :DQ].rearrange("p (h d) -> p h d", d=HD),
                        )

                    v_tasks.append(v_a)
                    v_tasks.append(v_b)

                yield from qk_tasks[0]
                yield from v_tasks
                yield from qk_tasks[1]

            def dma_task(tb):
                def t():
                    xblks[tb] = load_xblk(tb)

                return t

            def outproj_tasks(qb):
                qs = slice(qb * TQB, (qb + 1) * TQB)
                for n in range(NDC):

                    def t(n=n):
                        pso = psM.tile([P, TQB], f32, tag="mix", name="pso")
                        nc.tensor.matmul(
                            pso[:], woa_sb[:, n * P : (n + 1) * P],
                            ctxt[0][:, qs], start=True, stop=False,
                        )
                        nc.tensor.matmul(
                            pso[:], wob_sb[:, n * P : (n + 1) * P],
                            ctxt[1][:, qs], start=False, stop=True,
                        )
                        osb = obp.tile([P, TQB], f32, tag="osb", name="osb")
                        # alternate the PSUM->SBUF copy between DVE and the
                        # Scalar engine to balance engine load
                        if n % 2 == 0:
                            nc.vector.tensor_copy(osb[:], pso[:])
                        else:
                            nc.scalar.copy(osb[:], pso[:])
                        nc.sync.dma_start(ot[n * P : (n + 1) * P, qs], osb[:])

                    yield t

            def normalize_tasks(qb, g2, psc):
                """Normalise pass (qb, g2): l rows -> broadcast -> 1/l -> two
                multiplies into ctxt[g2] (even head rows 0-63, odd 64-127)."""
                qs = slice(qb * TQB, (qb + 1) * TQB)

                def t_bcast():
                    lrow = nrm.tile([P, 2 * TQB], f32, tag="lrow", name="lrow")
                    lb = nrm.tile([P, 2 * TQB], f32, tag="lb", name="lb")
                    rb = nrm.tile([P, 2 * TQB], f32, tag="rb", name="rb")
                    # gpsimd cannot read PSUM; stage the l row through SBUF
                    # (cross-partition copy 64 -> 0, validated on HW)
                    nc.vector.tensor_copy(lrow[0:1, :], psc[64:65, :])
                    nc.gpsimd.partition_broadcast(lb[0:64, 0:TQB], lrow[0:1, 0:TQB])
                    nc.gpsimd.partition_broadcast(lb[0:64, TQB:], lrow[0:1, TQB:])
                    nc.vector.reciprocal_approx_fast(rb[0:64, :], lb[0:64, :])
                    return rb

                state = {}

                def t0():
                    state["rb"] = t_bcast()

                def t1():
                    rb = state["rb"]
                    nc.vector.tensor_mul(
                        ctxt[g2][0:64, qs], psc[0:64, 0:TQB], rb[0:64, 0:TQB]
                    )

                def t2():
                    rb = state["rb"]
                    tmp = nrm.tile([P, TQB], bf, tag="ntmp", name="ntmp")
                    nc.vector.tensor_mul(tmp[0:64, :], psc[0:64, TQB:], rb[0:64, TQB:])
                    nc.vector.tensor_copy(ctxt[g2][64:128, qs], tmp[0:64, :])

                return [t0, t1, t2]

            def attention_pass(qb, g2, aux):
                """Causal attention for heads (2*g2, 2*g2+1) over query block
                qb. aux tasks are dripped between chunk groups."""
                nchunks = 4 * (qb + 1)
                q0 = qb * TQB
                psc = psC.tile([P, 2 * TQB], f32, tag="ctx", name="psc")
                pts = {}

                def scores_exp(ck):
                    tk0 = ck * P
                    co = max(0, tk0 - q0)
                    pss = psS.tile([P, 2 * TQB], f32, tag="scores", name="pss")
                    pt = ptp.tile([P, 2 * TQB], bf, tag="pt", name="pt")
                    for hh in range(2):
                        nc.tensor.matmul(
                            pss[:, hh * TQB + co : (hh + 1) * TQB],
                            kt[g2][hh * 64 : (hh + 1) * 64, tk0 : tk0 + P],
                            qt[g2][hh * 64 : (hh + 1) * 64, q0 + co : q0 + TQB],
                            start=True,
                            stop=True,
                        )
                    src3 = pss[:].rearrange("p (h q) -> p h q", h=2)[:, :, co:TQB]
                    gctr["n"] += 1
                    if DVE_EXP_MOD and gctr["n"] % DVE_EXP_MOD == 0:
                        nc.vector.tensor_scalar(
                            pt[:].bitcast(i16).rearrange("p (h q) -> p h q", h=2)[
                                :, :, co:TQB
                            ],
                            src3,
                            FE_A,
                            FE_B,
                            op0=ALU.mult,
                            op1=ALU.add,
                        )
                    else:
                        nc.scalar.activation(
                            pt[:].rearrange("p (h q) -> p h q", h=2)[:, :, co:TQB],
                            src3,
                            AF.Exp,
                            scale=1.0 / math.sqrt(HD),
                        )
                    if tk0 >= q0:
                        # diagonal chunk: zero the upper-left triangle via a
                        # DVE multiply with the lower-tri keep mask
                        for hh in range(2):
                            tgt = pt[:, hh * TQB + co : hh * TQB + co + P]
                            nc.vector.tensor_mul(tgt, tgt, tri_sb[:])
                    pts[ck] = (pt, co)

                def pv(ck):
                    pt, co = pts.pop(ck)
                    tc_ = ck
                    for hh in range(2):
                        h = 2 * g2 + hh
                        nc.tensor.matmul(
                            psc[0:65, hh * TQB + co : (hh + 1) * TQB],
                            vsb[:, tc_ * VST + h * 66 : tc_ * VST + h * 66 + 65],
                            pt[:, hh * TQB + co : (hh + 1) * TQB],
                            start=(ck == 0),
                            stop=(ck == nchunks - 1),
                        )

                emitted = 0
                done = 0
                total = nchunks + 1
                for ck in range(nchunks):
                    scores_exp(ck)
                    if ck >= LAG:
                        pv(ck - LAG)
                    done += 1
                    want = (done * len(aux)) // total
                    while emitted < want:
                        aux[emitted]()
                        emitted += 1
                for ck in range(max(0, nchunks - LAG), nchunks):
                    pv(ck)
                while emitted < len(aux):
                    aux[emitted]()
                    emitted += 1
                return psc

            # ---- software-pipelined schedule ----
            import contextlib

            _hint = (
                mybir.EngineType.PE,
                mybir.EngineType.Activation,
                mybir.EngineType.DVE,
                mybir.EngineType.Pool,
                mybir.EngineType.SP,
            )
            loop_cm = (
                tc.For_i(0, loop, 1, hint_engines=_hint)
                if loop > 1
                else contextlib.nullcontext()
            )
            with loop_cm:
              for _rep in range(repeat):
                dma_task(0)()
                dma_task(1)()
                for t in proj_tasks(0):
                    t()
                prev_norm = None  # normalize tasks of the previous pass
                for qb in range(NQB):
                    for g2 in range(2):
                        aux = []
                        if prev_norm is not None:
                            aux += prev_norm
                        if g2 == 0:
                            if qb + 1 < NQB:
                                pj = list(proj_tasks(qb + 1))
                                # Q j0 + K j0 first half
                                aux += pj[0:4]
                                rest = pj[4:]
                            else:
                                rest = []
                            if qb > 0:
                                aux += list(outproj_tasks(qb - 1))
                            state_rest = rest
                        else:
                            aux += state_rest
                            if qb + 2 < NQB:
                                aux.append(dma_task(qb + 2))
                        psc = attention_pass(qb, g2, aux)
                        prev_norm = normalize_tasks(qb, g2, psc)
                for t in prev_norm:
                    t()
                for t in outproj_tasks(NQB - 1):
                    t()

    nc.compile()
    return nc


def _get_nc():
    if "nc" not in _NC_CACHE:
        _NC_CACHE["nc"] = _build_nc()
    return _NC_CACHE["nc"]


def _in_maps(x, Wq, bq, Wk, bk, Wv, bv, Wo, bo):
    import ml_dtypes

    bf = ml_dtypes.bfloat16
    maps = []
    for c in range(NCORES):
        b, hg = divmod(c, 4)
        cs = slice(hg * DQ, (hg + 1) * DQ)
        bqk_pack = np.stack(
            [
                bq[cs][0:128], bq[cs][128:256],
                bk[cs][0:128], bk[cs][128:256],
            ],
            axis=1,
        ).astype(np.float32)
        maps.append(
            {
                "xT": np.ascontiguousarray(x[b].T).astype(bf),
                "wq": Wq[:, cs].astype(bf),
                "wk": Wk[:, cs].astype(bf),
                "wv": Wv[:, cs].astype(bf),
                "woa": Wo[cs, :][0:128].astype(bf),
                "wob": Wo[cs, :][128:256].astype(bf),
                "bqk": np.ascontiguousarray(bqk_pack),
                "bv1": bv[cs].reshape(1, DQ).astype(bf),
            }
        )
    return maps


def kernel(x, Wq, bq, Wk, bk, Wv, bv, Wo, bo):
    from concourse.bass_utils import run_bass_kernel_spmd

    nc = _get_nc()
    maps = _in_maps(x, Wq, bq, Wk, bk, Wv, bv, Wo, bo)
    res = run_bass_kernel_spmd(nc, maps, list(range(NCORES)))
    out = np.zeros((B, T, D), np.float32)
    for b in range(B):
        acc = res.results[b * 4]["ot"].astype(np.float32)
        for g in range(1, 4):
            acc = acc + res.results[b * 4 + g]["ot"]
        out[b] = acc.T + bo.astype(np.float32)
    return out
